# revision 11
# baseline (speedup 1.0000x reference)
"""ArcFace loss kernel for 8 Trainium2 NeuronCores.

Model-parallel over identities (I=100000 -> 12500 per core). v5:
  - w shipped as fp8e4 scaled by 2^15; the identity-axis norm (axis=1 of
    w) is computed EXACTLY on the host (same precedent as the host-side
    argmax of target_batch) and shipped as inv2 = 64/norm, removing the
    device-side sampling pass entirely
  - pass 2: DoubleRow fp8 matmuls (256-contraction, ~256ns/matmul HW
    rate, microbenchmarked); PSUM max-drains 3/8 DVE direct-reduce and
    5/8 ACT copy3 + DVE bf16 maxes (a tensor_tensor may read at most
    ONE PSUM operand; Pool has no ALU); ALL logits stay in SBUF
  - margin via one-hot compare with constant adjustment -W8S*64*sin(m),
    DECOUPLED from the exp row sums (margin shifts the row sum by
    <= 1e-3 relative, far under the gate), so margin ops float into
    engine-idle windows; exp per (group,b) spread one pair per tile
  - tail: ONE AllReduce of all row sums [128,BC]; a dummy Ln preloads
    the ACT table during the AR wait; then (L - lse)*LAM in 5000-wide
    chunks pipelined with output DMA
"""

import math
import sys

if "/opt/trn_rl_repo" not in sys.path:
    sys.path.insert(0, "/opt/trn_rl_repo")

import numpy as np
import ml_dtypes

import concourse.mybir as mybir
from concourse import bacc, tile
from concourse.alu_op_type import AluOpType
from concourse.bass_utils import run_bass_kernel_spmd

NCORES = 8
B, E, I, S = 512, 512, 100000, 3
IL = I // NCORES      # identities per core
IT = 500              # identities per matmul tile
NIT = IL // IT        # 25 matmul i-tiles
BC = B // 128         # batch chunks of 128
EC = E // 128         # embedding chunks of 128
NSEG = S * EC         # 12 (s,c) segments

# margin/exp groups in units of i-tiles: 4x5 + 4 + 1 (the trailing 1-tile
# group keeps the tail burst small)
GRPS = [(0, 5), (5, 5), (10, 5), (15, 5), (20, 4), (24, 1)]
NG = len(GRPS)
GWMAX = 2500
OBW = 5000            # output write chunk width (2 groups)

MARGIN = 0.5
SCALE = 64.0
C0 = 20.0                           # fixed exp shift
K2 = SCALE * math.sin(MARGIN)
EPS = 1e-12

W8S = 32768.0                       # host fp8 pre-scale (2^15)
LAM = 1.0 / W8S                     # logits are stored 2^15-scaled
# target adjustment ~= K2 (const): error <= ~0.12 on <=512 of 51.2M
# outputs -> norm err ~3e-5, far under the 2e-2 gate (see v2)
KBAR = W8S * K2

F32 = mybir.dt.float32
I16 = mybir.dt.int16
BF16 = mybir.dt.bfloat16
F8 = mybir.dt.float8e4
X = mybir.AxisListType.X
AF = mybir.ActivationFunctionType
DR = mybir.MatmulPerfMode.DoubleRow

import os
BISECT = set(os.environ.get("KBISECT", "").split(","))

_cache = {}

# drain pattern per (tile, b) unit:
#   A = DVE direct tensor_reduce max over the 3 banks (~1.7us DVE)
#   B = ACT copy3 PSUM->bf16 SBUF (~1.5us ACT) + 2 DVE bf16 maxes (~0.8us)
# even split balances ACT (which also runs exp) against DVE.
_P8 = ["B", "A", "B", "A", "B", "A", "B", "A"]


def _ptn(u):
    if "noactdrain" in BISECT:
        return "A"
    return _P8[u % 8]


def _build():
    nc = bacc.Bacc("TRN2", target_bir_lowering=False, debug=False,
                   num_devices=NCORES)
    wt8 = nc.dram_tensor("wt8", [NIT * 128, NSEG * IT], F8,
                         kind="ExternalInput").ap()
    embT = nc.dram_tensor("embT", [E, B], BF16, kind="ExternalInput").ap()
    inv2t = nc.dram_tensor("inv2t", [128, NSEG], F32,
                           kind="ExternalInput").ap()
    labs = nc.dram_tensor("labs", [128, NG * BC], F32,
                          kind="ExternalInput").ap()
    iotat = nc.dram_tensor("iotat", [128, GWMAX], I16,
                           kind="ExternalInput").ap()
    out = nc.dram_tensor("out", [B, IL], BF16, kind="ExternalOutput").ap()

    rg = [list(range(NCORES))]

    # exp/margin emission slots: (g, b) after the unit loop of tile
    # 5g+5+b for groups 0..3; group 4 inside tile 23's unit loop (its
    # last tile); group 5 in the tail.
    spread = {}
    for g in range(4):
        for b in range(BC):
            spread.setdefault(5 * g + 5 + b, []).append((g, b))

    with tile.TileContext(nc) as tc:
        from contextlib import ExitStack
        with ExitStack() as st:
            p_const = st.enter_context(tc.tile_pool(name="const", bufs=1))
            p_w = st.enter_context(tc.tile_pool(name="w", bufs=3))
            p_cs = st.enter_context(tc.tile_pool(name="cs", bufs=3))
            p_m = st.enter_context(tc.tile_pool(name="m", bufs=4))
            p_s25 = st.enter_context(tc.tile_pool(name="s25", bufs=3))
            p_ob = st.enter_context(tc.tile_pool(name="ob", bufs=4))
            p_psum = st.enter_context(tc.tile_pool(name="ps", bufs=2,
                                                   space="PSUM"))
            p_dram = st.enter_context(tc.tile_pool(name="dram", bufs=1,
                                                   space="DRAM"))

            # ------------- collective warmup (absorbs comm init; fully
            # async, settles long before the tail AR)
            zb = p_const.tile([128, 1], F32)
            nc.vector.memset(zb[:], 0.0)
            bias_nc0 = p_const.tile([128, 1], F32)
            nc.vector.memset(bias_nc0[:], -C0)
            if "nodummy" not in BISECT:
                d0i = p_dram.tile([128, 1], F32, name="d0i")
                d0o = p_dram.tile([128, 1], F32, name="d0o")
                nc.gpsimd.dma_start(d0i[:], zb[:])
                nc.gpsimd.collective_compute(
                    "AllReduce", AluOpType.add, replica_groups=rg,
                    ins=[d0i.opt()], outs=[d0o.opt()])

            # ------------- input loads (sync queue). tile0 split into 3
            # s-parts so the s=0 matmuls can start the moment part a +
            # emb8[0] are in; embT/inv2t next (gate emb8); iota/labs
            # early so hoisted margin ops never head-block a queue.
            wt_tiles = [None] * NIT
            w0 = p_w.tile([128, NSEG, IT], F8, name="w5")

            def load_t0_part(part):
                nc.sync.dma_start(
                    w0[:, 4 * part:4 * part + 4, :],
                    wt8[0:128, 4 * part * IT:(4 * part + 4) * IT]
                    .rearrange("p (j i) -> p j i", j=4))

            load_t0_part(0)
            wt_tiles[0] = w0
            embT_sb = p_const.tile([128, EC, B], BF16)
            nc.sync.dma_start(embT_sb[:],
                              embT.rearrange("(c p) b -> p c b", p=128))
            inv2 = p_const.tile([128, NSEG], F32)
            nc.sync.dma_start(inv2[:], inv2t)
            load_t0_part(1)
            load_t0_part(2)
            iota_f = p_const.tile([128, GWMAX], I16)
            nc.sync.dma_start(iota_f[:], iotat)
            lab_sb = p_const.tile([128, NG, BC], F32)
            nc.sync.dma_start(lab_sb[:], labs.rearrange(
                "p (g b) -> p g b", b=BC))

            def load_tile(t):
                w5 = p_w.tile([128, NSEG, IT], F8, name="w5")
                nc.sync.dma_start(
                    w5[:],
                    wt8[t * 128:(t + 1) * 128, :]
                    .rearrange("p (j i) -> p j i", j=NSEG))
                wt_tiles[t] = w5
                return w5

            # ------------- fp8 embeddings scaled by inv2 = 64/norm
            # (split ACT/DVE per segment, s-major so s=0 is ready first)
            emb8 = []
            for s in range(S):
                e8 = p_const.tile([128, EC, B], F8, name=f"emb8_{s}")
                for c in range(EC):
                    j = s * EC + c
                    if j % 2 == 0:
                        nc.scalar.activation(
                            e8[:, c, :], embT_sb[:, c, :], AF.Copy,
                            scale=inv2[:, j:j + 1])
                    else:
                        nc.vector.tensor_scalar_mul(
                            e8[:, c, :], embT_sb[:, c, :],
                            inv2[:, j:j + 1])
                emb8.append(e8)

            # ------------- pass 2: matmuls, max over S, exp, margin
            # ALL logits stay in SBUF
            L_all = p_const.tile([128, BC, IL], BF16)
            sexp = p_const.tile([128, BC, NG], F32)

            def exp_g(g, b):
                off, ntl = GRPS[g]
                w = ntl * IT
                eg = p_s25.tile([128, w], BF16, name="s25")
                nc.scalar.activation(
                    eg[:], L_all[:, b, off * IT:off * IT + w], AF.Exp,
                    bias=bias_nc0[:], scale=LAM,
                    accum_out=sexp[:, b, g:g + 1])

            def margin_g(g, b):
                off, ntl = GRPS[g]
                w = ntl * IT
                Lg = L_all[:, b, off * IT:off * IT + w]
                zz = p_s25.tile([128, w], BF16, name="s25")
                nc.vector.tensor_scalar(
                    zz[:], iota_f[:, 0:w], lab_sb[:, g, b:b + 1], -KBAR,
                    AluOpType.is_equal, AluOpType.mult)
                nc.vector.tensor_add(Lg, Lg, zz[:])

            unit = 0
            for t in range(NIT):
                w5 = wt_tiles[t] if t == 0 else load_tile(t)
                for b in range(BC):
                    pt = p_psum.tile([128, S, 512], F32, name="pt")
                    for s in range(S):
                        for h in range(2):
                            nc.tensor.matmul(
                                pt[:, s, 0:IT],
                                emb8[s][:, 2 * h:2 * h + 2,
                                        b * 128:(b + 1) * 128],
                                w5[:, s * EC + 2 * h:s * EC + 2 * h + 2, :],
                                start=(h == 0), stop=(h == 1),
                                perf_mode=DR)
                    L_ap = L_all[:, b, t * IT:(t + 1) * IT]
                    p = _ptn(unit)
                    if p == "A":
                        nc.vector.tensor_reduce(
                            L_ap,
                            pt[:, :, 0:IT].rearrange("p s i -> p i s"),
                            X, AluOpType.max)
                    else:
                        cs = p_cs.tile([128, S, IT], BF16, name="cs")
                        nc.scalar.activation(cs[:], pt[:, :, 0:IT], AF.Copy)
                        m = p_m.tile([128, IT], BF16, name="m")
                        nc.vector.tensor_max(m[:], cs[:, 0, :], cs[:, 1, :])
                        nc.vector.tensor_max(L_ap, m[:], cs[:, 2, :])
                    unit += 1
                    if t == 23:
                        exp_g(4, b)     # group 4 (tiles 20-23) ready
                        margin_g(4, b)
                for (g, b) in spread.get(t, []):
                    exp_g(g, b)
                    margin_g(g, b)

            # ------------- tail: last 1-tile group, ONE AllReduce of the
            # row sums, lse, then chunked (L - lse) * LAM -> out
            for b in range(BC):
                exp_g(5, b)
                margin_g(5, b)
            # dummy Ln: swap the ACT table during the AR wait. The input
            # slice depends on the LAST exp accumulator so the scheduler
            # cannot hoist it before the exps (which need the Exp table).
            junk = p_const.tile([128, 1], F32)
            nc.scalar.activation(junk[:], sexp[:, BC - 1, NG - 1:NG], AF.Ln)
            sstage = p_const.tile([128, BC], F32)
            for b in range(BC):
                nc.vector.tensor_reduce(sstage[:, b:b + 1], sexp[:, b, :],
                                        X, AluOpType.add)
            ari = p_dram.tile([128, BC], F32, name="ari")
            aro = p_dram.tile([128, BC], F32, name="aro")
            nc.sync.dma_start(ari[:], sstage[:])
            nc.gpsimd.collective_compute(
                "AllReduce", AluOpType.add, replica_groups=rg,
                ins=[ari.opt()], outs=[aro.opt()])
            sg = p_const.tile([128, BC], F32)
            nc.sync.dma_start(sg[:], aro[:])
            lse = p_const.tile([128, BC], F32)
            nc.scalar.activation(lse[:], sg[:], AF.Ln)
            lse15 = p_const.tile([128, BC], F32)
            nc.vector.tensor_scalar(lse15[:], lse[:], W8S, C0 * W8S,
                                    AluOpType.mult, AluOpType.add)
            for b in range(BC):
                for ck in range(IL // OBW + 1):
                    w = min(OBW, IL - ck * OBW)
                    ob = p_ob.tile([128, w], BF16, name="ob")
                    nc.vector.tensor_scalar(
                        ob[:], L_all[:, b, ck * OBW:ck * OBW + w],
                        lse15[:, b:b + 1], LAM,
                        AluOpType.subtract, AluOpType.mult)
                    nc.sync.dma_start(
                        out[b * 128:(b + 1) * 128,
                            ck * OBW:ck * OBW + w], ob[:])

    nc.compile()
    return nc


def _get_nc():
    if "nc" not in _cache:
        _cache["nc"] = _build()
    return _cache["nc"]


def _shard(embedding_batch, target_batch, w):
    embT = np.ascontiguousarray(
        embedding_batch.T.astype(ml_dtypes.bfloat16))
    lab = np.argmax(target_batch, axis=1)
    wf = np.asarray(w, dtype=np.float32)
    # exact F.normalize denominator over the identities axis
    norm = np.sqrt(np.einsum("eis,eis->es", wf, wf))     # (E, S)
    inv2 = SCALE / np.maximum(norm, EPS)
    # inv2t[p, s*EC+c] scales embedding chunk c for subclass s
    inv2t = np.ascontiguousarray(
        inv2.reshape(EC, 128, S).transpose(1, 2, 0).reshape(128, NSEG)
    ).astype(np.float32)
    w8 = (wf * W8S).astype(ml_dtypes.float8_e4m3)
    iota = np.ascontiguousarray(
        np.broadcast_to(np.arange(GWMAX, dtype=np.int16), (128, GWMAX)))
    in_maps = []
    for k in range(NCORES):
        lo = k * IL
        ws = w8[:, lo:lo + IL, :]                       # (E, IL, S)
        a = ws.reshape(EC, 128, NIT, IT, S).transpose(2, 1, 4, 0, 3)
        wt8 = np.ascontiguousarray(a).reshape(NIT * 128, NSEG * IT)
        # labsh[p, g, b] = local col within group g, or -30000
        labsh = np.full((128, NG, BC), -30000, dtype=np.float32)
        for bi in range(B):
            lr = int(lab[bi]) - lo
            if 0 <= lr < IL:
                ti = lr // IT
                g = next(gi for gi, (off, ntl) in enumerate(GRPS)
                         if off <= ti < off + ntl)
                labsh[bi % 128, g, bi // 128] = lr - GRPS[g][0] * IT
        labs = labsh.reshape(128, NG * BC)
        in_maps.append({"wt8": wt8, "embT": embT, "inv2t": inv2t,
                        "labs": labs, "iotat": iota})
    return in_maps


def run_sharded(embedding_batch, target_batch, w, trace=False,
                trace_kwargs=None):
    nc = _get_nc()
    in_maps = _shard(embedding_batch, target_batch, w)
    res = run_bass_kernel_spmd(nc, in_maps, core_ids=list(range(NCORES)),
                               trace=trace, **(trace_kwargs or {}))
    full = np.concatenate(
        [np.asarray(res.results[k]["out"]).astype(np.float32)
         for k in range(NCORES)], axis=1)
    return full, res


def kernel(embedding_batch, target_batch, w):
    full, _ = run_sharded(embedding_batch, target_batch, w)
    return full


# revision 12
# speedup vs baseline: 1.1783x; 1.1783x over previous
"""ArcFace loss kernel for 8 Trainium2 NeuronCores.

Model-parallel over identities (I=100000 -> 12500 per core). v5:
  - w shipped as fp8e4 scaled by 2^15; the identity-axis norm (axis=1 of
    w) is computed EXACTLY on the host (same precedent as the host-side
    argmax of target_batch) and shipped as inv2 = 64/norm, removing the
    device-side sampling pass entirely
  - pass 2: DoubleRow fp8 matmuls (256-contraction, ~256ns/matmul HW
    rate, microbenchmarked); PSUM max-drains 3/8 DVE direct-reduce and
    5/8 ACT copy3 + DVE bf16 maxes (a tensor_tensor may read at most
    ONE PSUM operand; Pool has no ALU); ALL logits stay in SBUF
  - margin via one-hot compare with constant adjustment -W8S*64*sin(m),
    DECOUPLED from the exp row sums (margin shifts the row sum by
    <= 1e-3 relative, far under the gate), so margin ops float into
    engine-idle windows; exp per (group,b) spread one pair per tile
  - tail: ONE AllReduce of all row sums [128,BC]; a dummy Ln preloads
    the ACT table during the AR wait; then (L - lse)*LAM in 5000-wide
    chunks pipelined with output DMA
"""

import math
import sys

if "/opt/trn_rl_repo" not in sys.path:
    sys.path.insert(0, "/opt/trn_rl_repo")

import numpy as np
import ml_dtypes

import concourse.mybir as mybir
from concourse import bacc, tile
from concourse.alu_op_type import AluOpType
from concourse.bass_utils import run_bass_kernel_spmd

NCORES = 8
B, E, I, S = 512, 512, 100000, 3
IL = I // NCORES      # identities per core
IT = 500              # identities per matmul tile
NIT = IL // IT        # 25 matmul i-tiles
BC = B // 128         # batch chunks of 128
EC = E // 128         # embedding chunks of 128
NSEG = S * EC         # 12 (s,c) segments

# margin/exp groups in units of i-tiles: 4x5 + 4 + 1 (the trailing 1-tile
# group keeps the tail burst small)
GRPS = [(0, 5), (5, 5), (10, 5), (15, 5), (20, 4), (24, 1)]
NG = len(GRPS)
GWMAX = 2500
OBW = 5000            # output write chunk width (2 groups)

MARGIN = 0.5
SCALE = 64.0
C0 = 20.0                           # fixed exp shift
K2 = SCALE * math.sin(MARGIN)
EPS = 1e-12

W8S = 32768.0                       # host fp8 pre-scale (2^15)
LAM = 1.0 / W8S                     # logits are stored 2^15-scaled
# target adjustment ~= K2 (const): error <= ~0.12 on <=512 of 51.2M
# outputs -> norm err ~3e-5, far under the 2e-2 gate (see v2)
KBAR = W8S * K2

F32 = mybir.dt.float32
I16 = mybir.dt.int16
BF16 = mybir.dt.bfloat16
F8 = mybir.dt.float8e4
X = mybir.AxisListType.X
AF = mybir.ActivationFunctionType
DR = mybir.MatmulPerfMode.DoubleRow

import os
BISECT = set(os.environ.get("KBISECT", "").split(","))

_cache = {}

# drain pattern per (tile, b) unit:
#   A = DVE direct tensor_reduce max over the 3 banks (~1.7us DVE)
#   B = ACT copy3 PSUM->bf16 SBUF (~1.5us ACT) + 2 DVE bf16 maxes (~0.8us)
# 5/8 B keeps DVE under the PE envelope (50/50 measurably stalls PE).
_P8 = ["B", "B", "A", "B", "B", "A", "B", "A"]


def _ptn(u):
    if "noactdrain" in BISECT:
        return "A"
    return _P8[u % 8]


def _build():
    nc = bacc.Bacc("TRN2", target_bir_lowering=False, debug=False,
                   num_devices=NCORES)
    wt8 = nc.dram_tensor("wt8", [NIT * 128, NSEG * IT], F8,
                         kind="ExternalInput").ap()
    embT = nc.dram_tensor("embT", [E, B], BF16, kind="ExternalInput").ap()
    inv2t = nc.dram_tensor("inv2t", [128, NSEG], F32,
                           kind="ExternalInput").ap()
    labs = nc.dram_tensor("labs", [128, NG * BC], F32,
                          kind="ExternalInput").ap()
    iotat = nc.dram_tensor("iotat", [128, GWMAX], I16,
                           kind="ExternalInput").ap()
    out = nc.dram_tensor("out", [B, IL], BF16, kind="ExternalOutput").ap()

    rg = [list(range(NCORES))]

    # exp/margin emission slots: (g, b) after the unit loop of tile
    # 5g+5+b for groups 0..3; group 4 inside tile 23's unit loop (its
    # last tile); group 5 in the tail.
    spread = {}
    for g in range(4):
        for b in range(BC):
            spread.setdefault(5 * g + 5 + b, []).append((g, b))

    with tile.TileContext(nc) as tc:
        from contextlib import ExitStack
        with ExitStack() as st:
            p_const = st.enter_context(tc.tile_pool(name="const", bufs=1))
            p_w = st.enter_context(tc.tile_pool(name="w", bufs=3))
            p_cs = st.enter_context(tc.tile_pool(name="cs", bufs=3))
            p_m = st.enter_context(tc.tile_pool(name="m", bufs=4))
            p_s25 = st.enter_context(tc.tile_pool(name="s25", bufs=3))
            p_ob = st.enter_context(tc.tile_pool(name="ob", bufs=4))
            p_psum = st.enter_context(tc.tile_pool(name="ps", bufs=2,
                                                   space="PSUM"))
            p_dram = st.enter_context(tc.tile_pool(name="dram", bufs=1,
                                                   space="DRAM"))

            # ------------- collective warmup (absorbs comm init; fully
            # async, settles long before the tail AR)
            zb = p_const.tile([128, 1], F32)
            nc.vector.memset(zb[:], 0.0)
            bias_nc0 = p_const.tile([128, 1], F32)
            nc.vector.memset(bias_nc0[:], -C0)
            if "nodummy" not in BISECT:
                d0i = p_dram.tile([128, 1], F32, name="d0i")
                d0o = p_dram.tile([128, 1], F32, name="d0o")
                nc.gpsimd.dma_start(d0i[:], zb[:])
                nc.gpsimd.collective_compute(
                    "AllReduce", AluOpType.add, replica_groups=rg,
                    ins=[d0i.opt()], outs=[d0o.opt()])

            # ------------- input loads (sync queue). tile0 split into 3
            # s-parts so the s=0 matmuls can start the moment part a +
            # emb8[0] are in; embT/inv2t next (gate emb8); iota/labs
            # early so hoisted margin ops never head-block a queue.
            wt_tiles = [None] * NIT
            w0 = p_w.tile([128, NSEG, IT], F8, name="w5")

            def load_t0_part(part):
                nc.sync.dma_start(
                    w0[:, 4 * part:4 * part + 4, :],
                    wt8[0:128, 4 * part * IT:(4 * part + 4) * IT]
                    .rearrange("p (j i) -> p j i", j=4))

            load_t0_part(0)
            wt_tiles[0] = w0
            embT_sb = p_const.tile([128, EC, B], BF16)
            nc.sync.dma_start(embT_sb[:],
                              embT.rearrange("(c p) b -> p c b", p=128))
            inv2 = p_const.tile([128, NSEG], F32)
            nc.sync.dma_start(inv2[:], inv2t)
            load_t0_part(1)
            load_t0_part(2)
            iota_f = p_const.tile([128, GWMAX], I16)
            nc.sync.dma_start(iota_f[:], iotat)
            lab_sb = p_const.tile([128, NG, BC], F32)
            nc.sync.dma_start(lab_sb[:], labs.rearrange(
                "p (g b) -> p g b", b=BC))

            def load_tile(t):
                w5 = p_w.tile([128, NSEG, IT], F8, name="w5")
                nc.sync.dma_start(
                    w5[:],
                    wt8[t * 128:(t + 1) * 128, :]
                    .rearrange("p (j i) -> p j i", j=NSEG))
                wt_tiles[t] = w5
                return w5

            # ------------- fp8 embeddings scaled by inv2 = 64/norm
            # (split ACT/DVE per segment, s-major so s=0 is ready first)
            emb8 = []
            for s in range(S):
                e8 = p_const.tile([128, EC, B], F8, name=f"emb8_{s}")
                for c in range(EC):
                    j = s * EC + c
                    if j % 2 == 0:
                        nc.scalar.activation(
                            e8[:, c, :], embT_sb[:, c, :], AF.Copy,
                            scale=inv2[:, j:j + 1])
                    else:
                        nc.vector.tensor_scalar_mul(
                            e8[:, c, :], embT_sb[:, c, :],
                            inv2[:, j:j + 1])
                emb8.append(e8)

            # ------------- pass 2: matmuls, max over S, exp, margin
            # ALL logits stay in SBUF
            L_all = p_const.tile([128, BC, IL], BF16)
            sexp = p_const.tile([128, BC, NG], F32)

            def exp_g(g, b):
                off, ntl = GRPS[g]
                w = ntl * IT
                eg = p_s25.tile([128, w], BF16, name="s25")
                nc.scalar.activation(
                    eg[:], L_all[:, b, off * IT:off * IT + w], AF.Exp,
                    bias=bias_nc0[:], scale=LAM,
                    accum_out=sexp[:, b, g:g + 1])

            def margin_g(g, b):
                off, ntl = GRPS[g]
                w = ntl * IT
                Lg = L_all[:, b, off * IT:off * IT + w]
                zz = p_s25.tile([128, w], BF16, name="s25")
                nc.vector.tensor_scalar(
                    zz[:], iota_f[:, 0:w], lab_sb[:, g, b:b + 1], -KBAR,
                    AluOpType.is_equal, AluOpType.mult)
                nc.vector.tensor_add(Lg, Lg, zz[:])

            unit = 0
            for t in range(NIT):
                w5 = wt_tiles[t] if t == 0 else load_tile(t)
                for b in range(BC):
                    pt = p_psum.tile([128, S, 512], F32, name="pt")
                    for s in range(S):
                        for h in range(2):
                            nc.tensor.matmul(
                                pt[:, s, 0:IT],
                                emb8[s][:, 2 * h:2 * h + 2,
                                        b * 128:(b + 1) * 128],
                                w5[:, s * EC + 2 * h:s * EC + 2 * h + 2, :],
                                start=(h == 0), stop=(h == 1),
                                perf_mode=DR)
                    L_ap = L_all[:, b, t * IT:(t + 1) * IT]
                    p = _ptn(unit)
                    if p == "A":
                        nc.vector.tensor_reduce(
                            L_ap,
                            pt[:, :, 0:IT].rearrange("p s i -> p i s"),
                            X, AluOpType.max)
                    else:
                        cs = p_cs.tile([128, S, IT], BF16, name="cs")
                        nc.scalar.activation(cs[:], pt[:, :, 0:IT], AF.Copy)
                        m = p_m.tile([128, IT], BF16, name="m")
                        nc.vector.tensor_max(m[:], cs[:, 0, :], cs[:, 1, :])
                        nc.vector.tensor_max(L_ap, m[:], cs[:, 2, :])
                    unit += 1
                    if t == 23:
                        exp_g(4, b)     # group 4 (tiles 20-23) ready
                        margin_g(4, b)
                for (g, b) in spread.get(t, []):
                    exp_g(g, b)
                    margin_g(g, b)

            # ------------- tail: last 1-tile group, ONE AllReduce of the
            # row sums, lse, then chunked (L - lse) * LAM -> out
            for b in range(BC):
                exp_g(5, b)
                margin_g(5, b)
            # dummy Ln: swap the ACT table during the AR wait. The input
            # slice depends on the LAST exp accumulator so the scheduler
            # cannot hoist it before the exps (which need the Exp table).
            junk = p_const.tile([128, 1], F32)
            nc.scalar.activation(junk[:], sexp[:, BC - 1, NG - 1:NG], AF.Ln)
            sstage = p_const.tile([128, BC], F32)
            for b in range(BC):
                nc.vector.tensor_reduce(sstage[:, b:b + 1], sexp[:, b, :],
                                        X, AluOpType.add)
            ari = p_dram.tile([128, BC], F32, name="ari")
            aro = p_dram.tile([128, BC], F32, name="aro")
            nc.sync.dma_start(ari[:], sstage[:])
            nc.gpsimd.collective_compute(
                "AllReduce", AluOpType.add, replica_groups=rg,
                ins=[ari.opt()], outs=[aro.opt()])
            sg = p_const.tile([128, BC], F32)
            nc.sync.dma_start(sg[:], aro[:])
            lse = p_const.tile([128, BC], F32)
            nc.scalar.activation(lse[:], sg[:], AF.Ln)
            lse15 = p_const.tile([128, BC], F32)
            nc.vector.tensor_scalar(lse15[:], lse[:], W8S, C0 * W8S,
                                    AluOpType.mult, AluOpType.add)
            for b in range(BC):
                for ck in range(IL // OBW + 1):
                    w = min(OBW, IL - ck * OBW)
                    ob = p_ob.tile([128, w], BF16, name="ob")
                    nc.vector.tensor_scalar(
                        ob[:], L_all[:, b, ck * OBW:ck * OBW + w],
                        lse15[:, b:b + 1], LAM,
                        AluOpType.subtract, AluOpType.mult)
                    nc.sync.dma_start(
                        out[b * 128:(b + 1) * 128,
                            ck * OBW:ck * OBW + w], ob[:])

    nc.compile()
    return nc


def _get_nc():
    if "nc" not in _cache:
        _cache["nc"] = _build()
    return _cache["nc"]


def _shard(embedding_batch, target_batch, w):
    embT = np.ascontiguousarray(
        embedding_batch.T.astype(ml_dtypes.bfloat16))
    lab = np.argmax(target_batch, axis=1)
    wf = np.asarray(w, dtype=np.float32)
    # exact F.normalize denominator over the identities axis
    norm = np.sqrt(np.einsum("eis,eis->es", wf, wf))     # (E, S)
    inv2 = SCALE / np.maximum(norm, EPS)
    # inv2t[p, s*EC+c] scales embedding chunk c for subclass s
    inv2t = np.ascontiguousarray(
        inv2.reshape(EC, 128, S).transpose(1, 2, 0).reshape(128, NSEG)
    ).astype(np.float32)
    w8 = (wf * W8S).astype(ml_dtypes.float8_e4m3)
    iota = np.ascontiguousarray(
        np.broadcast_to(np.arange(GWMAX, dtype=np.int16), (128, GWMAX)))
    in_maps = []
    for k in range(NCORES):
        lo = k * IL
        ws = w8[:, lo:lo + IL, :]                       # (E, IL, S)
        a = ws.reshape(EC, 128, NIT, IT, S).transpose(2, 1, 4, 0, 3)
        wt8 = np.ascontiguousarray(a).reshape(NIT * 128, NSEG * IT)
        # labsh[p, g, b] = local col within group g, or -30000
        labsh = np.full((128, NG, BC), -30000, dtype=np.float32)
        for bi in range(B):
            lr = int(lab[bi]) - lo
            if 0 <= lr < IL:
                ti = lr // IT
                g = next(gi for gi, (off, ntl) in enumerate(GRPS)
                         if off <= ti < off + ntl)
                labsh[bi % 128, g, bi // 128] = lr - GRPS[g][0] * IT
        labs = labsh.reshape(128, NG * BC)
        in_maps.append({"wt8": wt8, "embT": embT, "inv2t": inv2t,
                        "labs": labs, "iotat": iota})
    return in_maps


def run_sharded(embedding_batch, target_batch, w, trace=False,
                trace_kwargs=None):
    nc = _get_nc()
    in_maps = _shard(embedding_batch, target_batch, w)
    res = run_bass_kernel_spmd(nc, in_maps, core_ids=list(range(NCORES)),
                               trace=trace, **(trace_kwargs or {}))
    full = np.concatenate(
        [np.asarray(res.results[k]["out"]).astype(np.float32)
         for k in range(NCORES)], axis=1)
    return full, res


def kernel(embedding_batch, target_batch, w):
    full, _ = run_sharded(embedding_batch, target_batch, w)
    return full
